# revision 14
# baseline (speedup 1.0000x reference)
"""MultiHeadAttention (B=2, S=2048, HID=1024, NH=16, HD=64, RoPE) on 8 TRN2 cores.

Sharding: 8 cores = 2 batches x 4 head-groups (4 heads per core).
Per core: q/k/v projections for its 4 heads (tensor parallel on H), RoPE,
attention, and a partial o-projection over its 256 channels. Host sums the
4 partial o-projections per batch (the TP unshard) and adds bo.

Layouts (host-prepared, per core):
  xT   [1024, 2048]  x[b].T                      (contraction dim on partitions)
  wqT/wkT/wvT [1024, 256]  W[g].T                (per-head-group slices)
  woT  [256, 1024]   wo[:, g].T  in bf16         ([c, o] layout)
  cosT/sinT [128, 2048]  RoPE tables for a 2-head partition tile; sinT carries
      the rotate-half signs so RoPE is: q_rot = q*cosT + shift32(q)*sinT,
      where shift32 swaps 32-row halves inside each 64-row head block.

Device pipeline per core:
  1. q/k projections in [c, s] layout; RoPE fused into PSUM eviction
     (the 32-row shift via ACT cross-partition-window copies reading PSUM).
  2. v projection in [s, c] layout, stored bf16 with a ones-column per head.
  3. scores^T = k_rot^T.T @ q_rot^T per (head, q-chunk, k-tile); exp to bf16:
     most tiles via ScalarE Exp, a tunable subset via a Schraudolph bit-trick
     on the Vector engine (engine load balancing; softmax normalization
     cancels the common-mode exp error).
  4. AV in flipped orientation: p-tile [k,q] is the stationary operand, v
     columns (plus a ones column producing the softmax row-sum) stream; out
     is [q, d] in PSUM so normalization is a per-partition broadcast multiply.
  5. PE-transpose of the normalized [q, c] tile back to [c, q] (bf16), then
     partial o-projection -> out [2048, 1024].

Projections/scores matmuls run float32r (full PE rate at N>=512); AV and the
o-projection run bf16.
"""

import numpy as np

B, S, HID = 2, 2048, 1024
NH, HD = 16, 64
BASE = 10000.0
N_CORES = 8
GROUPS = 4                 # head groups (tensor parallel)
HPC = NH // GROUPS         # heads per core = 4
CPC = HPC * HD             # channels per core = 256
SC = 512                   # seq chunk (matmul free dim)
NSC = S // SC              # 4
NST = S // 128             # 16 s-tiles / k-tiles
KO = HID // 128            # 8 contraction slices for projections
HD1 = HD + 1               # head block incl. ones column

# exp bit-trick constants: exp(0.125*x) ~= bitcast_bf16(int16(x*EA + EB))
_LOG2E = 1.4426950408889634
EXP_A = (1 << 23) * _LOG2E * 0.125 / 65536.0
EXP_B = (127 * (1 << 23) - 366393.0) / 65536.0
# k-tiles whose exp runs on the Vector engine (single-tile granules with
# their own PSUM, so the 2-slot score-PSUM ring stays purely ACT-paced)
DVE_EXP_KTS = (4, 5, 10, 11)

_cached = None


def _split_waits(nc, mybir, limit=1):
    """This walrus build accepts at most one embedded sync wait per
    instruction; hoist the rest onto NoOps just before it on the same engine."""
    n = 0
    for f in nc.m.functions:
        for b in f.blocks:
            out = []
            changed = False
            for inst in b.instructions:
                si = inst.sync_info
                waits = list(si.on_wait) if (si and si.on_wait) else []
                if len(waits) > limit:
                    keep = waits[-limit:]
                    excess = waits[:-limit]
                    for ci in range(0, len(excess), limit):
                        out.append(mybir.InstNoOp(
                            name=f"{inst.name}-wsplit-{ci}",
                            engine=inst.engine,
                            sync_info=mybir.SyncInfo(
                                on_wait=excess[ci:ci + limit], on_update=[]),
                            bass_nofuse=True,
                        ))
                        n += 1
                    inst.sync_info = mybir.SyncInfo(
                        on_wait=keep,
                        on_update=(list(si.on_update) if si else []))
                    changed = True
                out.append(inst)
            if changed:
                b.instructions = out
    return n


def _build():
    import concourse.bass as bass
    import concourse.mybir as mybir
    import concourse.tile as tile
    from concourse.masks import make_identity

    f32 = mybir.dt.float32
    f32r = mybir.dt.float32r
    bf16 = mybir.dt.bfloat16
    i16 = mybir.dt.int16

    nc = bass.Bass()
    xT = nc.dram_tensor("xT", [HID, S], f32r, kind="ExternalInput")
    wqT = nc.dram_tensor("wqT", [HID, CPC], f32r, kind="ExternalInput")
    wkT = nc.dram_tensor("wkT", [HID, CPC], f32r, kind="ExternalInput")
    wvT = nc.dram_tensor("wvT", [HID, CPC], f32r, kind="ExternalInput")
    woT = nc.dram_tensor("woT", [CPC, HID], bf16, kind="ExternalInput")
    cosT = nc.dram_tensor("cosT", [128, S], f32, kind="ExternalInput")
    sinT = nc.dram_tensor("sinT", [128, S], f32, kind="ExternalInput")
    out = nc.dram_tensor("out", [S, HID], f32, kind="ExternalOutput")

    with tile.TileContext(nc) as tc:
        with (
            tc.tile_pool(name="persist", bufs=1) as persist,
            tc.tile_pool(name="ptmp", bufs=2) as ptmp,
        ):
            # ---- persistent SBUF ---- (DMAs for cos/sin/wo are emitted
            # after the first x/wk transfers; see phase A)
            cos_sb = persist.tile([128, S], f32)
            sin_sb = persist.tile([128, S], f32)
            wo_sb = persist.tile([128, 2, HID], bf16)
            # q_rot/k_rot in [c, s]; V in [s, c] bf16 with a ones column/head
            q_rot = [persist.tile([128, S], f32r, name=f"qrot{i}") for i in range(2)]
            k_rot = [persist.tile([128, S], f32r, name=f"krot{i}") for i in range(2)]
            v_sb = persist.tile([128, NST, HPC * HD1], bf16)
            vcols = v_sb[:].rearrange("p t (h e) -> p t h e", e=HD1)
            nc.vector.memset(vcols[:, :, :, HD], 1.0)
            ident = persist.tile([128, 128], bf16)
            make_identity(nc, ident[:])

            # ---- phase A: projections + RoPE ----
            with (
                tc.tile_pool(name="xw", bufs=1) as xw,
                tc.tile_pool(name="pv", bufs=2, space="PSUM") as pv_pool,
                tc.tile_pool(name="pqk", bufs=3, space="PSUM") as pqk_pool,
            ):
                x_sb = [xw.tile([128, S], f32r, name=f"x{ko}") for ko in range(KO)]
                wk_sb = [xw.tile([128, CPC], f32r, name=f"wk{ko}") for ko in range(KO)]
                wq_sb = [xw.tile([128, CPC], f32r, name=f"wq{ko}") for ko in range(KO)]
                wv_sb = [xw.tile([128, CPC], f32r, name=f"wv{ko}") for ko in range(KO)]
                def dma_w(w_sb, wdram):
                    for ko in range(KO):
                        nc.sync.dma_start(
                            w_sb[ko][:], wdram[ko * 128:(ko + 1) * 128, :])
                def dma_x(quarter):
                    hs = slice(quarter * SC, (quarter + 1) * SC)
                    for ko in range(KO):
                        nc.sync.dma_start(
                            x_sb[ko][:, hs], xT[ko * 128:(ko + 1) * 128, hs])
                dma_w(wk_sb, wkT)
                dma_x(0)
                nc.sync.dma_start(cos_sb[:], cosT[:])
                nc.sync.dma_start(sin_sb[:], sinT[:])
                dma_x(1)
                dma_w(wq_sb, wqT)
                dma_w(wv_sb, wvT)
                for cs in range(2):
                    nc.sync.dma_start(wo_sb[:, cs],
                                      woT[cs * 128:(cs + 1) * 128, :])
                dma_x(2)
                dma_x(3)

                # k then q projections in [c, s] with fused RoPE eviction;
                # mt=0 (heads 0,1) first so attention can start early.
                # v-proj s-tiles are interleaved to fill PE time while the
                # DVE drains RoPE evictions.
                def qk_chunk(w_sb, rot, mt, ntp):
                    # chunk-pair: psum [128, 2, SC] (2 banks), evictions on
                    # [*, 2*SC] to amortize DVE per-op overhead
                    ps = pqk_pool.tile([128, 2, SC], f32, name="pqk")
                    for half in range(2):
                        nt = ntp * 2 + half
                        for ko in range(KO):
                            nc.tensor.matmul(
                                ps[:, half],
                                w_sb[ko][:, mt * 128:(mt + 1) * 128],
                                x_sb[ko][:, nt * SC:(nt + 1) * SC],
                                start=(ko == 0), stop=(ko == KO - 1),
                            )
                    sl = slice(ntp * 2 * SC, (ntp + 1) * 2 * SC)
                    cs2 = cos_sb[:, sl].rearrange("p (a s) -> p a s", a=2)
                    sn2 = sin_sb[:, sl].rearrange("p (a s) -> p a s", a=2)
                    # rotate-half shift via ACT cross-partition copies (ACT is
                    # idle in phase A); muls full-width on DVE; add on Pool.
                    qs = ptmp.tile([128, 2, SC], f32, tag="qs")
                    for blk in range(4):
                        o0 = blk * 32
                        i0 = (blk ^ 1) * 32
                        nc.scalar.copy(out=qs[o0:o0 + 32], in_=ps[i0:i0 + 32])
                    qc = ptmp.tile([128, 2, SC], f32, tag="qc")
                    nc.vector.tensor_mul(out=qc[:], in0=ps[:], in1=cs2)
                    nc.vector.tensor_mul(out=qs[:], in0=qs[:], in1=sn2)
                    rot2 = rot[mt][:, sl].rearrange("p (a s) -> p a s", a=2)
                    nc.gpsimd.tensor_add(out=rot2, in0=qc[:], in1=qs[:])

                def v_tile(st):
                    ps = pv_pool.tile([128, CPC], f32, name="pv")
                    for ko in range(KO):
                        nc.tensor.matmul(
                            ps[:],
                            x_sb[ko][:, st * 128:(st + 1) * 128],
                            wv_sb[ko][:],
                            start=(ko == 0), stop=(ko == KO - 1),
                        )
                    psv = ps[:].rearrange("p (h e) -> p h e", e=HD)
                    nc.vector.tensor_copy(out=vcols[:, st, :, 0:HD], in_=psv[:])

                # emission follows DMA arrival: half-0 chunks (k then q,
                # both m-tiles), v s-tiles 0-7, then half-1, v s-tiles 8-15
                for ntp, vlo in ((0, 0), (1, 8)):
                    for w_sb, rot in ((wk_sb, k_rot), (wq_sb, q_rot)):
                        for mt in range(2):
                            qk_chunk(w_sb, rot, mt, ntp)
                            if w_sb is wq_sb:
                                for st in range(vlo + mt * 4,
                                                vlo + mt * 4 + 4):
                                    v_tile(st)

            # ---- phase B+C: attention, software-pipelined ----
            # Per step (q-chunk, head): score tiles + exp are emitted
            # interleaved with the previous step's AV accumulation blocks so
            # PE work fills the exp-drain windows instead of racing ahead of
            # the 2-slot score PSUM rotation. Transposes (DMA xbar) and
            # o-projection s-tiles are deferred units drained 2 per step.
            with (
                tc.tile_pool(name="pb", bufs=2) as pb,
                tc.tile_pool(name="pc", bufs=2) as pc,
                tc.tile_pool(name="ps_pool", bufs=2, space="PSUM") as ps_pool,
                tc.tile_pool(name="pav", bufs=2, space="PSUM") as pav_pool,
                tc.tile_pool(name="po", bufs=2, space="PSUM") as po_pool,
            ):
                def make_avt(qc_i):
                    return pc.tile([128, NSC, CPC], bf16, tag="avt",
                                   name=f"avt{qc_i}")

                def av_block(h, p_sb, avp, qt):
                    for kt in range(NST):
                        nc.tensor.matmul(
                            avp[:, qt],
                            p_sb[:, kt, qt * 128:(qt + 1) * 128],
                            v_sb[:, kt, h * HD1:(h + 1) * HD1],
                            start=(kt == 0), stop=(kt == NST - 1),
                        )

                def norm(h, avp, avt):
                    rec = ptmp.tile([128, NSC, 1], f32, tag="rec")
                    nc.vector.reciprocal(out=rec[:], in_=avp[:, :, HD:HD + 1])
                    nc.vector.tensor_mul(
                        out=avt[:, :, h * HD:(h + 1) * HD],
                        in0=avp[:, :, 0:HD],
                        in1=rec[:].broadcast_to([128, NSC, HD]),
                    )

                def transp_unit(qc_i, avt, box):
                    def emit():
                        avtT = pc.tile([128, 2, SC], bf16, tag="avtT",
                                       name=f"avtT{qc_i}")
                        for qt in range(NSC):
                            tp = po_pool.tile([128, 2, 128], bf16, name="po",
                                              tag="po")
                            for ch in range(2):
                                nc.tensor.transpose(
                                    tp[:, ch],
                                    avt[:, qt, ch * 128:(ch + 1) * 128],
                                    ident[:])
                            nc.vector.tensor_copy(
                                out=avtT[:, :, qt * 128:(qt + 1) * 128],
                                in_=tp[:])
                        box.append(avtT)
                    return emit

                def oproj_unit(qc_i, sti, box):
                    def emit():
                        avtT = box[0]
                        st = qc_i * 4 + sti
                        o_sb = pc.tile([128, 2 * SC], f32, tag="o_sb")
                        for oc in range(2):
                            po = po_pool.tile([128, SC], f32, name="po",
                                              tag="po")
                            for cs in range(2):
                                nc.tensor.matmul(
                                    po[:],
                                    avtT[:, cs, sti * 128:(sti + 1) * 128],
                                    wo_sb[:, cs, oc * SC:(oc + 1) * SC],
                                    start=(cs == 0), stop=(cs == 1),
                                )
                            nc.vector.tensor_copy(
                                out=o_sb[:, oc * SC:(oc + 1) * SC], in_=po[:])
                        nc.sync.dma_start(
                            out[st * 128:(st + 1) * 128, :], o_sb[:])
                    return emit

                deferred = []
                AV_AT = {1: 0, 3: 1, 5: 2, 7: 3}  # ktg -> AV qt of prev step
                DEF_AT = (3, 7)

                def finish_prev(pstep, pp, pavp, avt_prev):
                    pqc, ph = pstep
                    norm(ph, pavp, avt_prev)
                    if ph == HPC - 1:
                        box = []
                        deferred.append(transp_unit(pqc, avt_prev, box))
                        for sti in range(4):
                            deferred.append(oproj_unit(pqc, sti, box))

                steps = [(qc_i, h) for qc_i in range(NSC) for h in range(HPC)]
                prev = None
                avt_cur = None
                for step in steps:
                    qc_i, h = step
                    tl, pof = h // 2, (h % 2) * 64
                    qsl = slice(qc_i * SC, (qc_i + 1) * SC)
                    p_sb = pb.tile([128, NST, SC], bf16, tag="p_sb",
                                   name=f"p{qc_i}_{h}")
                    if h == 0:
                        avt_next = make_avt(qc_i)
                    pavp = (pav_pool.tile([128, NSC, HD1], f32, tag="avp",
                                          name="avp")
                            if prev is not None else None)
                    # per-step schedule: ACT score-tile pairs on the sps
                    # ring, DVE exp granules on the po pool, AV blocks and
                    # deferred transpose/o-proj units filling PE stall windows
                    sched = (
                        ("act", 0), ("act", 1), ("av", 0), ("dve", 4),
                        ("act", 3), ("av", 1), ("dve", 5), ("pop",),
                        ("act", 4), ("av", 2), ("dve", 10),
                        ("act", 6), ("act", 7), ("av", 3), ("dve", 11),
                        ("pop",),
                    )
                    for item in sched:
                        kind = item[0]
                        if kind == "act":
                            ktg = item[1]
                            sps = ps_pool.tile([128, 2, SC], f32, name="sps")
                            for kti in range(2):
                                kt = ktg * 2 + kti
                                nc.tensor.matmul(
                                    sps[:, kti],
                                    k_rot[tl][pof:pof + HD,
                                              kt * 128:(kt + 1) * 128],
                                    q_rot[tl][pof:pof + HD, qsl],
                                    start=True, stop=True,
                                )
                            nc.scalar.activation(
                                out=p_sb[:, ktg * 2:(ktg + 1) * 2], in_=sps[:],
                                func=mybir.ActivationFunctionType.Exp,
                                scale=0.125,
                            )
                        elif kind == "dve":
                            kt = item[1]
                            dps = po_pool.tile([128, SC], f32, name="dps",
                                               tag="po")
                            nc.tensor.matmul(
                                dps[:],
                                k_rot[tl][pof:pof + HD,
                                          kt * 128:(kt + 1) * 128],
                                q_rot[tl][pof:pof + HD, qsl],
                                start=True, stop=True,
                            )
                            nc.vector.tensor_scalar(
                                out=p_sb[:, kt].bitcast(i16), in0=dps[:],
                                scalar1=EXP_A, scalar2=EXP_B,
                                op0=mybir.AluOpType.mult,
                                op1=mybir.AluOpType.add,
                            )
                        elif kind == "av":
                            if prev is not None:
                                av_block(prev[0][1], prev[1], pavp, item[1])
                        elif kind == "pop":
                            if deferred:
                                deferred.pop(0)()
                    if prev is not None:
                        finish_prev(prev[0], prev[1], pavp, avt_cur)
                    if h == 0:
                        avt_cur = avt_next
                    prev = (step, p_sb)

                # drain: last step's AV + norm + transpose + o-projection
                pavp = pav_pool.tile([128, NSC, HD1], f32, tag="avp",
                                     name="avp_last")
                for qt in range(NSC):
                    av_block(prev[0][1], prev[1], pavp, qt)
                finish_prev(prev[0], prev[1], pavp, avt_cur)
                while deferred:
                    deferred.pop(0)()

    _split_waits(nc, mybir)
    return nc


def _rope_tables():
    inv_freq = 1.0 / (BASE ** (np.arange(0, HD, 2, dtype=np.float32) / HD))
    t = np.arange(S, dtype=np.float32)
    freqs = np.einsum("i,j->ij", t, inv_freq)        # [S, 32]
    emb = np.concatenate([freqs, freqs], axis=-1)    # [S, 64]
    cos = np.cos(emb).T.astype(np.float32)           # [64, S]
    sin = np.sin(emb).T.astype(np.float32)
    sin_signed = np.concatenate([-sin[0:32], sin[32:64]], axis=0)
    cosT = np.tile(cos, (2, 1)).copy()               # [128, S]
    sinT = np.tile(sin_signed, (2, 1)).copy()
    return cosT, sinT


def _run(inputs, trace=False):
    global _cached
    import ml_dtypes
    from concourse.bass_utils import run_bass_kernel_spmd

    x = np.asarray(inputs["x"], dtype=np.float32)
    wq = np.asarray(inputs["wq"], dtype=np.float32)
    wk = np.asarray(inputs["wk"], dtype=np.float32)
    wv = np.asarray(inputs["wv"], dtype=np.float32)
    wo = np.asarray(inputs["wo"], dtype=np.float32)
    bq = np.asarray(inputs["bq"], dtype=np.float32)
    bk = np.asarray(inputs["bk"], dtype=np.float32)
    bv = np.asarray(inputs["bv"], dtype=np.float32)
    bo = np.asarray(inputs["bo"], dtype=np.float32)
    assert not (bq.any() or bk.any() or bv.any()), \
        "nonzero qkv biases not supported by this kernel build"

    if _cached is None:
        _cached = _build()
    nc = _cached

    cosT, sinT = _rope_tables()
    in_maps = []
    for core in range(N_CORES):
        b, g = divmod(core, GROUPS)
        cs = slice(g * CPC, (g + 1) * CPC)
        in_maps.append({
            "xT": np.ascontiguousarray(x[b].T),
            "wqT": np.ascontiguousarray(wq[cs].T),
            "wkT": np.ascontiguousarray(wk[cs].T),
            "wvT": np.ascontiguousarray(wv[cs].T),
            "woT": np.ascontiguousarray(wo[:, cs].T).astype(ml_dtypes.bfloat16),
            "cosT": cosT,
            "sinT": sinT,
        })

    res = run_bass_kernel_spmd(
        nc, in_maps, core_ids=list(range(N_CORES)), trace=trace)

    outp = np.zeros((B, S, HID), dtype=np.float32)
    for core in range(N_CORES):
        b = core // GROUPS
        outp[b] += res.results[core]["out"]
    outp += bo
    return outp, res


def kernel(**inputs):
    outp, _ = _run(inputs, trace=False)
    return outp


# revision 15
# speedup vs baseline: 1.0045x; 1.0045x over previous
"""MultiHeadAttention (B=2, S=2048, HID=1024, NH=16, HD=64, RoPE) on 8 TRN2 cores.

Sharding: 8 cores = 2 batches x 4 head-groups (4 heads per core).
Per core: q/k/v projections for its 4 heads (tensor parallel on H), RoPE,
attention, and a partial o-projection over its 256 channels. Host sums the
4 partial o-projections per batch (the TP unshard) and adds bo.

Layouts (host-prepared, per core):
  xT   [1024, 2048]  x[b].T                      (contraction dim on partitions)
  wqT/wkT/wvT [1024, 256]  W[g].T                (per-head-group slices)
  woT  [256, 1024]   wo[:, g].T  in bf16         ([c, o] layout)
  cosT/sinT [128, 2048]  RoPE tables for a 2-head partition tile; sinT carries
      the rotate-half signs so RoPE is: q_rot = q*cosT + shift32(q)*sinT,
      where shift32 swaps 32-row halves inside each 64-row head block.

Device pipeline per core:
  1. q/k projections in [c, s] layout; RoPE fused into PSUM eviction
     (the 32-row shift via ACT cross-partition-window copies reading PSUM).
  2. v projection in [s, c] layout, stored bf16 with a ones-column per head.
  3. scores^T = k_rot^T.T @ q_rot^T per (head, q-chunk, k-tile); exp to bf16:
     most tiles via ScalarE Exp, a tunable subset via a Schraudolph bit-trick
     on the Vector engine (engine load balancing; softmax normalization
     cancels the common-mode exp error).
  4. AV in flipped orientation: p-tile [k,q] is the stationary operand, v
     columns (plus a ones column producing the softmax row-sum) stream; out
     is [q, d] in PSUM so normalization is a per-partition broadcast multiply.
  5. PE-transpose of the normalized [q, c] tile back to [c, q] (bf16), then
     partial o-projection -> out [2048, 1024].

Projections/scores matmuls run float32r (full PE rate at N>=512); AV and the
o-projection run bf16.
"""

import numpy as np

B, S, HID = 2, 2048, 1024
NH, HD = 16, 64
BASE = 10000.0
N_CORES = 8
GROUPS = 4                 # head groups (tensor parallel)
HPC = NH // GROUPS         # heads per core = 4
CPC = HPC * HD             # channels per core = 256
SC = 512                   # seq chunk (matmul free dim)
NSC = S // SC              # 4
NST = S // 128             # 16 s-tiles / k-tiles
KO = HID // 128            # 8 contraction slices for projections
HD1 = HD + 1               # head block incl. ones column

# exp bit-trick constants: exp(0.125*x) ~= bitcast_bf16(int16(x*EA + EB))
_LOG2E = 1.4426950408889634
EXP_A = (1 << 23) * _LOG2E * 0.125 / 65536.0
EXP_B = (127 * (1 << 23) - 366393.0) / 65536.0
# k-tiles whose exp runs on the Vector engine (single-tile granules with
# their own PSUM, so the 2-slot score-PSUM ring stays purely ACT-paced)
DVE_EXP_KTS = (4, 5, 10, 11)

_cached = None


def _split_waits(nc, mybir, limit=1):
    """This walrus build accepts at most one embedded sync wait per
    instruction; hoist the rest onto NoOps just before it on the same engine."""
    n = 0
    for f in nc.m.functions:
        for b in f.blocks:
            out = []
            changed = False
            for inst in b.instructions:
                si = inst.sync_info
                waits = list(si.on_wait) if (si and si.on_wait) else []
                if len(waits) > limit:
                    keep = waits[-limit:]
                    excess = waits[:-limit]
                    for ci in range(0, len(excess), limit):
                        out.append(mybir.InstNoOp(
                            name=f"{inst.name}-wsplit-{ci}",
                            engine=inst.engine,
                            sync_info=mybir.SyncInfo(
                                on_wait=excess[ci:ci + limit], on_update=[]),
                            bass_nofuse=True,
                        ))
                        n += 1
                    inst.sync_info = mybir.SyncInfo(
                        on_wait=keep,
                        on_update=(list(si.on_update) if si else []))
                    changed = True
                out.append(inst)
            if changed:
                b.instructions = out
    return n


def _build():
    import concourse.bass as bass
    import concourse.mybir as mybir
    import concourse.tile as tile
    from concourse.masks import make_identity

    f32 = mybir.dt.float32
    f32r = mybir.dt.float32r
    bf16 = mybir.dt.bfloat16
    i16 = mybir.dt.int16

    nc = bass.Bass()
    xT = nc.dram_tensor("xT", [HID, S], f32r, kind="ExternalInput")
    wqT = nc.dram_tensor("wqT", [HID, CPC], f32r, kind="ExternalInput")
    wkT = nc.dram_tensor("wkT", [HID, CPC], f32r, kind="ExternalInput")
    wvT = nc.dram_tensor("wvT", [HID, CPC], f32r, kind="ExternalInput")
    woT = nc.dram_tensor("woT", [CPC, HID], bf16, kind="ExternalInput")
    cosT = nc.dram_tensor("cosT", [128, S], f32, kind="ExternalInput")
    sinT = nc.dram_tensor("sinT", [128, S], f32, kind="ExternalInput")
    out = nc.dram_tensor("out", [S, HID], f32, kind="ExternalOutput")

    with tile.TileContext(nc) as tc:
        with (
            tc.tile_pool(name="persist", bufs=1) as persist,
            tc.tile_pool(name="ptmp", bufs=2) as ptmp,
        ):
            # ---- persistent SBUF ---- (DMAs for cos/sin/wo are emitted
            # after the first x/wk transfers; see phase A)
            cos_sb = persist.tile([128, S], f32)
            sin_sb = persist.tile([128, S], f32)
            wo_sb = persist.tile([128, 2, HID], bf16)
            # q_rot/k_rot in [c, s]; V in [s, c] bf16 with a ones column/head
            q_rot = [persist.tile([128, S], f32r, name=f"qrot{i}") for i in range(2)]
            k_rot = [persist.tile([128, S], f32r, name=f"krot{i}") for i in range(2)]
            v_sb = persist.tile([128, NST, HPC * HD1], bf16)
            vcols = v_sb[:].rearrange("p t (h e) -> p t h e", e=HD1)
            nc.vector.memset(vcols[:, :, :, HD], 1.0)
            ident = persist.tile([128, 128], bf16)
            make_identity(nc, ident[:])

            # ---- phase A: projections + RoPE ----
            with (
                tc.tile_pool(name="xw", bufs=1) as xw,
                tc.tile_pool(name="pv", bufs=2, space="PSUM") as pv_pool,
                tc.tile_pool(name="pqk", bufs=3, space="PSUM") as pqk_pool,
            ):
                x_sb = [xw.tile([128, S], f32r, name=f"x{ko}") for ko in range(KO)]
                wk_sb = [xw.tile([128, CPC], f32r, name=f"wk{ko}") for ko in range(KO)]
                wq_sb = [xw.tile([128, CPC], f32r, name=f"wq{ko}") for ko in range(KO)]
                wv_sb = [xw.tile([128, CPC], f32r, name=f"wv{ko}") for ko in range(KO)]
                def dma_w(w_sb, wdram):
                    for ko in range(KO):
                        nc.sync.dma_start(
                            w_sb[ko][:], wdram[ko * 128:(ko + 1) * 128, :])
                def dma_x(quarter):
                    hs = slice(quarter * SC, (quarter + 1) * SC)
                    for ko in range(KO):
                        nc.sync.dma_start(
                            x_sb[ko][:, hs], xT[ko * 128:(ko + 1) * 128, hs])
                dma_w(wk_sb, wkT)
                dma_x(0)
                dma_x(1)
                nc.sync.dma_start(cos_sb[:], cosT[:])
                nc.sync.dma_start(sin_sb[:], sinT[:])
                dma_w(wq_sb, wqT)
                dma_w(wv_sb, wvT)
                for cs in range(2):
                    nc.sync.dma_start(wo_sb[:, cs],
                                      woT[cs * 128:(cs + 1) * 128, :])
                dma_x(2)
                dma_x(3)

                # k then q projections in [c, s] with fused RoPE eviction;
                # mt=0 (heads 0,1) first so attention can start early.
                # v-proj s-tiles are interleaved to fill PE time while the
                # DVE drains RoPE evictions.
                def qk_chunk(w_sb, rot, mt, ntp):
                    # chunk-pair: psum [128, 2, SC] (2 banks), evictions on
                    # [*, 2*SC] to amortize DVE per-op overhead
                    ps = pqk_pool.tile([128, 2, SC], f32, name="pqk")
                    for half in range(2):
                        nt = ntp * 2 + half
                        for ko in range(KO):
                            nc.tensor.matmul(
                                ps[:, half],
                                w_sb[ko][:, mt * 128:(mt + 1) * 128],
                                x_sb[ko][:, nt * SC:(nt + 1) * SC],
                                start=(ko == 0), stop=(ko == KO - 1),
                            )
                    sl = slice(ntp * 2 * SC, (ntp + 1) * 2 * SC)
                    cs2 = cos_sb[:, sl].rearrange("p (a s) -> p a s", a=2)
                    sn2 = sin_sb[:, sl].rearrange("p (a s) -> p a s", a=2)
                    # rotate-half shift via ACT cross-partition copies (ACT is
                    # idle in phase A); muls full-width on DVE; add on Pool.
                    qs = ptmp.tile([128, 2, SC], f32, tag="qs")
                    for blk in range(4):
                        o0 = blk * 32
                        i0 = (blk ^ 1) * 32
                        nc.scalar.copy(out=qs[o0:o0 + 32], in_=ps[i0:i0 + 32])
                    qc = ptmp.tile([128, 2, SC], f32, tag="qc")
                    nc.vector.tensor_mul(out=qc[:], in0=ps[:], in1=cs2)
                    nc.vector.tensor_mul(out=qs[:], in0=qs[:], in1=sn2)
                    rot2 = rot[mt][:, sl].rearrange("p (a s) -> p a s", a=2)
                    nc.gpsimd.tensor_add(out=rot2, in0=qc[:], in1=qs[:])

                def v_tile(st):
                    ps = pv_pool.tile([128, CPC], f32, name="pv")
                    for ko in range(KO):
                        nc.tensor.matmul(
                            ps[:],
                            x_sb[ko][:, st * 128:(st + 1) * 128],
                            wv_sb[ko][:],
                            start=(ko == 0), stop=(ko == KO - 1),
                        )
                    psv = ps[:].rearrange("p (h e) -> p h e", e=HD)
                    nc.vector.tensor_copy(out=vcols[:, st, :, 0:HD], in_=psv[:])

                # emission follows DMA arrival: half-0 chunks (k then q,
                # both m-tiles), v s-tiles 0-7, then half-1, v s-tiles 8-15
                for ntp, vlo in ((0, 0), (1, 8)):
                    for w_sb, rot in ((wk_sb, k_rot), (wq_sb, q_rot)):
                        for mt in range(2):
                            qk_chunk(w_sb, rot, mt, ntp)
                            if w_sb is wq_sb:
                                for st in range(vlo + mt * 4,
                                                vlo + mt * 4 + 4):
                                    v_tile(st)

            # ---- phase B+C: attention, software-pipelined ----
            # Per step (q-chunk, head): score tiles + exp are emitted
            # interleaved with the previous step's AV accumulation blocks so
            # PE work fills the exp-drain windows instead of racing ahead of
            # the 2-slot score PSUM rotation. Transposes (DMA xbar) and
            # o-projection s-tiles are deferred units drained 2 per step.
            with (
                tc.tile_pool(name="pb", bufs=2) as pb,
                tc.tile_pool(name="pc", bufs=2) as pc,
                tc.tile_pool(name="ps_pool", bufs=2, space="PSUM") as ps_pool,
                tc.tile_pool(name="pav", bufs=2, space="PSUM") as pav_pool,
                tc.tile_pool(name="po", bufs=2, space="PSUM") as po_pool,
            ):
                def make_avt(qc_i):
                    return pc.tile([128, NSC, CPC], bf16, tag="avt",
                                   name=f"avt{qc_i}")

                def av_block(h, p_sb, avp, qt):
                    for kt in range(NST):
                        nc.tensor.matmul(
                            avp[:, qt],
                            p_sb[:, kt, qt * 128:(qt + 1) * 128],
                            v_sb[:, kt, h * HD1:(h + 1) * HD1],
                            start=(kt == 0), stop=(kt == NST - 1),
                        )

                def norm(h, avp, avt):
                    rec = ptmp.tile([128, NSC, 1], f32, tag="rec")
                    nc.vector.reciprocal(out=rec[:], in_=avp[:, :, HD:HD + 1])
                    nc.vector.tensor_mul(
                        out=avt[:, :, h * HD:(h + 1) * HD],
                        in0=avp[:, :, 0:HD],
                        in1=rec[:].broadcast_to([128, NSC, HD]),
                    )

                def transp_unit(qc_i, avt, box):
                    def emit():
                        avtT = pc.tile([128, 2, SC], bf16, tag="avtT",
                                       name=f"avtT{qc_i}")
                        for qt in range(NSC):
                            tp = po_pool.tile([128, 2, 128], bf16, name="po",
                                              tag="po")
                            for ch in range(2):
                                nc.tensor.transpose(
                                    tp[:, ch],
                                    avt[:, qt, ch * 128:(ch + 1) * 128],
                                    ident[:])
                            nc.vector.tensor_copy(
                                out=avtT[:, :, qt * 128:(qt + 1) * 128],
                                in_=tp[:])
                        box.append(avtT)
                    return emit

                def oproj_unit(qc_i, sti, box):
                    def emit():
                        avtT = box[0]
                        st = qc_i * 4 + sti
                        o_sb = pc.tile([128, 2 * SC], f32, tag="o_sb")
                        for oc in range(2):
                            po = po_pool.tile([128, SC], f32, name="po",
                                              tag="po")
                            for cs in range(2):
                                nc.tensor.matmul(
                                    po[:],
                                    avtT[:, cs, sti * 128:(sti + 1) * 128],
                                    wo_sb[:, cs, oc * SC:(oc + 1) * SC],
                                    start=(cs == 0), stop=(cs == 1),
                                )
                            nc.vector.tensor_copy(
                                out=o_sb[:, oc * SC:(oc + 1) * SC], in_=po[:])
                        nc.sync.dma_start(
                            out[st * 128:(st + 1) * 128, :], o_sb[:])
                    return emit

                deferred = []
                AV_AT = {1: 0, 3: 1, 5: 2, 7: 3}  # ktg -> AV qt of prev step
                DEF_AT = (3, 7)

                def finish_prev(pstep, pp, pavp, avt_prev):
                    pqc, ph = pstep
                    norm(ph, pavp, avt_prev)
                    if ph == HPC - 1:
                        box = []
                        deferred.append(transp_unit(pqc, avt_prev, box))
                        for sti in range(4):
                            deferred.append(oproj_unit(pqc, sti, box))

                steps = [(qc_i, h) for qc_i in range(NSC) for h in range(HPC)]
                prev = None
                avt_cur = None
                for step in steps:
                    qc_i, h = step
                    tl, pof = h // 2, (h % 2) * 64
                    qsl = slice(qc_i * SC, (qc_i + 1) * SC)
                    p_sb = pb.tile([128, NST, SC], bf16, tag="p_sb",
                                   name=f"p{qc_i}_{h}")
                    if h == 0:
                        avt_next = make_avt(qc_i)
                    pavp = (pav_pool.tile([128, NSC, HD1], f32, tag="avp",
                                          name="avp")
                            if prev is not None else None)
                    # per-step schedule: ACT score-tile pairs on the sps
                    # ring, DVE exp granules on the po pool, AV blocks and
                    # deferred transpose/o-proj units filling PE stall windows
                    sched = (
                        ("act", 0), ("act", 1), ("av", 0), ("dve", 4),
                        ("act", 3), ("av", 1), ("dve", 5), ("pop",),
                        ("act", 4), ("av", 2), ("dve", 10),
                        ("act", 6), ("act", 7), ("av", 3), ("dve", 11),
                        ("pop",),
                    )
                    for item in sched:
                        kind = item[0]
                        if kind == "act":
                            ktg = item[1]
                            sps = ps_pool.tile([128, 2, SC], f32, name="sps")
                            for kti in range(2):
                                kt = ktg * 2 + kti
                                nc.tensor.matmul(
                                    sps[:, kti],
                                    k_rot[tl][pof:pof + HD,
                                              kt * 128:(kt + 1) * 128],
                                    q_rot[tl][pof:pof + HD, qsl],
                                    start=True, stop=True,
                                )
                            nc.scalar.activation(
                                out=p_sb[:, ktg * 2:(ktg + 1) * 2], in_=sps[:],
                                func=mybir.ActivationFunctionType.Exp,
                                scale=0.125,
                            )
                        elif kind == "dve":
                            kt = item[1]
                            dps = po_pool.tile([128, SC], f32, name="dps",
                                               tag="po")
                            nc.tensor.matmul(
                                dps[:],
                                k_rot[tl][pof:pof + HD,
                                          kt * 128:(kt + 1) * 128],
                                q_rot[tl][pof:pof + HD, qsl],
                                start=True, stop=True,
                            )
                            nc.vector.tensor_scalar(
                                out=p_sb[:, kt].bitcast(i16), in0=dps[:],
                                scalar1=EXP_A, scalar2=EXP_B,
                                op0=mybir.AluOpType.mult,
                                op1=mybir.AluOpType.add,
                            )
                        elif kind == "av":
                            if prev is not None:
                                av_block(prev[0][1], prev[1], pavp, item[1])
                        elif kind == "pop":
                            if deferred:
                                deferred.pop(0)()
                    if prev is not None:
                        finish_prev(prev[0], prev[1], pavp, avt_cur)
                    if h == 0:
                        avt_cur = avt_next
                    prev = (step, p_sb)

                # drain: last step's AV + norm + transpose + o-projection
                pavp = pav_pool.tile([128, NSC, HD1], f32, tag="avp",
                                     name="avp_last")
                for qt in range(NSC):
                    av_block(prev[0][1], prev[1], pavp, qt)
                finish_prev(prev[0], prev[1], pavp, avt_cur)
                while deferred:
                    deferred.pop(0)()

    _split_waits(nc, mybir)
    return nc


def _rope_tables():
    inv_freq = 1.0 / (BASE ** (np.arange(0, HD, 2, dtype=np.float32) / HD))
    t = np.arange(S, dtype=np.float32)
    freqs = np.einsum("i,j->ij", t, inv_freq)        # [S, 32]
    emb = np.concatenate([freqs, freqs], axis=-1)    # [S, 64]
    cos = np.cos(emb).T.astype(np.float32)           # [64, S]
    sin = np.sin(emb).T.astype(np.float32)
    sin_signed = np.concatenate([-sin[0:32], sin[32:64]], axis=0)
    cosT = np.tile(cos, (2, 1)).copy()               # [128, S]
    sinT = np.tile(sin_signed, (2, 1)).copy()
    return cosT, sinT


def _run(inputs, trace=False):
    global _cached
    import ml_dtypes
    from concourse.bass_utils import run_bass_kernel_spmd

    x = np.asarray(inputs["x"], dtype=np.float32)
    wq = np.asarray(inputs["wq"], dtype=np.float32)
    wk = np.asarray(inputs["wk"], dtype=np.float32)
    wv = np.asarray(inputs["wv"], dtype=np.float32)
    wo = np.asarray(inputs["wo"], dtype=np.float32)
    bq = np.asarray(inputs["bq"], dtype=np.float32)
    bk = np.asarray(inputs["bk"], dtype=np.float32)
    bv = np.asarray(inputs["bv"], dtype=np.float32)
    bo = np.asarray(inputs["bo"], dtype=np.float32)
    assert not (bq.any() or bk.any() or bv.any()), \
        "nonzero qkv biases not supported by this kernel build"

    if _cached is None:
        _cached = _build()
    nc = _cached

    cosT, sinT = _rope_tables()
    in_maps = []
    for core in range(N_CORES):
        b, g = divmod(core, GROUPS)
        cs = slice(g * CPC, (g + 1) * CPC)
        in_maps.append({
            "xT": np.ascontiguousarray(x[b].T),
            "wqT": np.ascontiguousarray(wq[cs].T),
            "wkT": np.ascontiguousarray(wk[cs].T),
            "wvT": np.ascontiguousarray(wv[cs].T),
            "woT": np.ascontiguousarray(wo[:, cs].T).astype(ml_dtypes.bfloat16),
            "cosT": cosT,
            "sinT": sinT,
        })

    res = run_bass_kernel_spmd(
        nc, in_maps, core_ids=list(range(N_CORES)), trace=trace)

    outp = np.zeros((B, S, HID), dtype=np.float32)
    for core in range(N_CORES):
        b = core // GROUPS
        outp[b] += res.results[core]["out"]
    outp += bo
    return outp, res


def kernel(**inputs):
    outp, _ = _run(inputs, trace=False)
    return outp
